# revision 19
# baseline (speedup 1.0000x reference)
"""Distribution tokenizer (per-row 64-bin histogram) for Trainium2, 8 NeuronCores.

Problem: x (32, 512, 1024) f32, boundaries (63,) f32 sorted ascending.
For every row (b, t): bin(x) = #{j : boundaries[j] <= x} (searchsorted right),
z[b, t, k] = count of bin k in the 1024-element feature row / 1024.

Algorithm (exact, no approximations):
  For each threshold j: H_j = #{f : x[f] >= b_j}. Then with Hext =
  [F, H_0, ..., H_62, 0], counts[k] = Hext[k] - Hext[k+1], z = counts / 1024
  (division by 2^10 is exact in fp32, counts are integers <= 1024).

  The 63 thresholds are split across two engines working in parallel:
   - DVE (vector): tensor_scalar(op0=is_ge, scalar1=b_j, op1=add reduce,
     accum_out=H_j) -- one fused mask+reduce instruction per threshold.
   - ACT (scalar): pass1 s = Sign(-x + b_j) (+1 iff x < b_j, 0 iff x == b_j),
     pass2 Relu(s) with accum_out = L_j = #{x < b_j}; H_j = F - L_j.
  All comparisons are exact fp32 comparator ops against the exact boundary
  values, so ties (x == b_j) are handled identically to searchsorted.

Sharding: pure data parallel, batch dim 32 -> 8 cores x 4.
"""

import os

import numpy as np

B, T, F = 32, 512, 1024
NB = 64            # number of bins
NTH = NB - 1       # number of thresholds (63)
N_CORES = 8
ROWS_PER_CORE = (B // N_CORES) * T        # 2048
P = 128                                   # SBUF partitions
N_TILES = ROWS_PER_CORE // P              # 16

# Threshold split: j in [0, N_DVE) on the vector engine, rest on scalar engine.
# Balanced from TimelineSim: DVE ~605ns/threshold/tile, ACT ~2.3us.
N_DVE = 50

_PROGRAM_CACHE = {}

_GE_PAIR_NAME = "GE_PAIR_ACC_ANT"


def _register_ge_pair():
    """Register a custom DVE op computing, per element,
    (x >= s0) + (x >= s1) * imm2, with accum_out = row sum.

    With imm2 = 4096 the accumulated value packs two threshold counts
    (each <= 1024 < 4096, sum < 2^22, exact in fp32) into one fused 1x
    pass -- two thresholds per 1024-element read instead of one.
    """
    from operator import add as _add

    import concourse.dve_ops as dve_ops
    from concourse.dve_spec import C0, C1, C2, Spec, Src0, lower
    from concourse.dve_uop import DveOpSpec

    if _GE_PAIR_NAME in dve_ops._SUB_OPCODE_FOR_NAME:
        for op in dve_ops.OPS:
            if op.name == _GE_PAIR_NAME:
                return op

    body = (Src0 >= C0) + (Src0 >= C1) * C2

    def ref(in0, in1, s0, s1, imm2):
        b = (
            (in0.astype(np.float32) >= s0).astype(np.float32)
            + (in0.astype(np.float32) >= s1).astype(np.float32) * imm2
        ).astype(np.float32)
        return b, b.reshape(b.shape[0], -1).sum(axis=-1, keepdims=True)

    spec = Spec(body=body, accum=_add, reference=ref)
    shas = {}
    for ver in ("v3", "v4"):
        tmp = DveOpSpec(name=_GE_PAIR_NAME, opcode=31, uops=lower(spec, ver=ver),
                        rd1_en=False)
        shas[ver] = tmp.sha(ver)
    op = dve_ops.DveOp(_GE_PAIR_NAME, spec, subdim=False, uops_sha=shas)
    dve_ops.OPS.append(op)
    dve_ops.CUSTOM_DVE_SPECS[_GE_PAIR_NAME] = spec
    dve_ops._SUB_OPCODE_FOR_NAME[_GE_PAIR_NAME] = (
        max(dve_ops._SUB_OPCODE_FOR_NAME.values()) + 1
    )
    return op


def _build_program(bvals, repeat=1):
    """Build the per-core Bass program. bvals: list of 63 exact float values.

    repeat>1 re-runs the whole tile loop (perf slope measurement only).
    """
    import concourse.bass as bass
    import concourse.mybir as mybir
    import concourse.tile as tile
    from concourse import bacc

    f32 = mybir.dt.float32
    bf16 = mybir.dt.bfloat16
    Alu = mybir.AluOpType
    Act = mybir.ActivationFunctionType

    # Bacc (not raw Bass): its compile() runs generate_event_semaphores,
    # which splits multi-wait instructions to satisfy the TRN2 limit of
    # one sync wait per instruction.
    nc = bacc.Bacc("TRN2")
    x_d = nc.dram_tensor("x", [ROWS_PER_CORE, F], f32, kind="ExternalInput")
    z_d = nc.dram_tensor("z", [ROWS_PER_CORE, NB], f32, kind="ExternalOutput")

    assert N_DVE % 2 == 0
    n_pairs = N_DVE // 2
    n_act = NTH - N_DVE
    ge_pair = _register_ge_pair()

    # Register const [P,1] APs for ACT bias values (boundaries used on ACT and
    # the row total F), exactly like Bass.__init__ does for 0.0/1.0. These are
    # written before the TileContext so tile scheduling sees them as plain
    # constant reads with no tracked writers.
    def register_const(value):
        key = (f32, value)
        if key not in nc.const_aps.aps:
            t = nc.alloc_sbuf_tensor(f"const-f32-{value}", [P, 1], f32)
            nc.gpsimd.memset(t.ap(), value)
            nc.const_aps.aps[key] = t.ap()

    for j in range(N_DVE, NTH):
        register_const(bvals[j])
    register_const(float(F))
    nc.all_engine_barrier()

    with tile.TileContext(nc) as tc:
        with (
            tc.tile_pool(name="xp", bufs=3) as xp,
            tc.tile_pool(name="hp", bufs=2) as hp,
            tc.tile_pool(name="lp", bufs=2) as lp,
            tc.tile_pool(name="hp2", bufs=2) as hp2,
            tc.tile_pool(name="sp", bufs=2) as sp,
            tc.tile_pool(name="tv", bufs=2) as tv,
            tc.tile_pool(name="pp", bufs=2) as pp,
            tc.tile_pool(name="rp", bufs=2) as rp,
            tc.tile_pool(name="rp2", bufs=2) as rp2,
            tc.tile_pool(name="ts", bufs=2) as ts,
            tc.tile_pool(name="zp", bufs=2) as zp,
        ):
            for i in [t for _ in range(repeat) for t in range(N_TILES)]:
                xt = xp.tile([P, F], f32)
                nc.sync.dma_start(xt[:], x_d[bass.ts(i, P), :])

                hext = hp.tile([P, NB + 1], f32)
                nc.vector.memset(hext[:, 0:1], float(F))
                nc.vector.memset(hext[:, NB:NB + 1], 0.0)

                # DVE: packed pairs. Pair p counts thresholds p and p+n_pairs
                # in one fused 1x pass: accum = H_p + 4096*H_{p+n_pairs}.
                trash_v = tv.tile([P, F], bf16)
                pbuf = pp.tile([P, max(n_pairs, 1)], f32)
                for p in range(n_pairs):
                    nc.vector._custom_dve(
                        ge_pair, out=trash_v[:], in0=xt[:],
                        s0=bvals[p], s1=bvals[p + n_pairs], imm2=4096.0,
                        accum_out=pbuf[:, p:p + 1],
                    )
                # Unpack: hi = RNE(P/4096) via the 2^23 trick (frac <= 0.25
                # always rounds down), lo = P - 4096*hi. All exact in fp32.
                rbuf = rp.tile([P, max(n_pairs, 1)], f32)
                nc.vector.tensor_scalar(
                    rbuf[:], pbuf[:], float(2.0 ** -12), float(2.0 ** 23),
                    Alu.mult, Alu.add,
                )
                nc.vector.tensor_scalar(
                    hext[:, 1 + n_pairs:1 + 2 * n_pairs], rbuf[:],
                    float(2.0 ** 23), None, Alu.subtract,
                )
                sbuf = rp2.tile([P, max(n_pairs, 1)], f32)
                nc.vector.tensor_scalar(
                    sbuf[:], rbuf[:], float(2.0 ** 23), 4096.0,
                    Alu.subtract, Alu.mult,
                )
                nc.vector.tensor_tensor(
                    hext[:, 1:1 + n_pairs], pbuf[:], sbuf[:], Alu.subtract,
                )

                if n_act:
                    lbuf = lp.tile([P, n_act], f32)
                    for k in range(n_act):
                        j = N_DVE + k
                        sgn = sp.tile([P, F], bf16)
                        nc.scalar.activation(
                            sgn[:], xt[:], Act.Sign,
                            bias=bvals[j], scale=-1.0,
                        )
                        trash_s = ts.tile([P, F], bf16)
                        nc.scalar.activation(
                            trash_s[:], sgn[:], Act.Relu,
                            accum_out=lbuf[:, k:k + 1],
                        )
                    # H_j = F - L_j, ACT-side into an ACT-owned tile; a single
                    # DVE copy then moves it into hext. Every cross-engine
                    # handoff tile has exactly one writer instruction (more
                    # blows the per-instruction sync-wait limit in codegen).
                    hact = hp2.tile([P, n_act], f32)
                    nc.scalar.activation(
                        hact[:], lbuf[:], Act.Identity,
                        bias=float(F), scale=-1.0,
                    )
                    nc.vector.tensor_copy(hext[:, 1 + N_DVE:1 + NTH], hact[:])

                zt = zp.tile([P, NB], f32)
                nc.vector.tensor_tensor(
                    zt[:], hext[:, 0:NB], hext[:, 1:NB + 1], Alu.subtract,
                )
                nc.vector.tensor_scalar(
                    zt[:], zt[:], float(2.0 ** -10), None, Alu.mult,
                )
                nc.sync.dma_start(z_d[bass.ts(i, P), :], zt[:])

    if not nc.is_finalized():
        nc.finalize()
    return nc


def _get_program(b):
    key = b.tobytes()
    if key not in _PROGRAM_CACHE:
        _PROGRAM_CACHE[key] = _build_program([float(v) for v in b])
    return _PROGRAM_CACHE[key]


def run(x, boundaries, trace=False):
    """Run on hardware; returns (z, BassKernelResults)."""
    from concourse.bass_utils import run_bass_kernel_spmd

    x = np.ascontiguousarray(np.asarray(x), dtype=np.float32)
    b = np.ascontiguousarray(np.asarray(boundaries), dtype=np.float32)
    assert x.shape == (B, T, F) and b.shape == (NTH,)

    nc = _get_program(b)
    bpc = B // N_CORES
    in_maps = [
        {"x": np.ascontiguousarray(x[c * bpc:(c + 1) * bpc].reshape(ROWS_PER_CORE, F))}
        for c in range(N_CORES)
    ]
    res = run_bass_kernel_spmd(nc, in_maps, core_ids=list(range(N_CORES)), trace=trace)
    z = np.stack([res.results[c]["z"].reshape(bpc, T, NB) for c in range(N_CORES)])
    return z.reshape(B, T, NB), res


def kernel(x, boundaries, nr_of_bins):
    assert int(nr_of_bins) == NB
    z, _ = run(x, boundaries)
    return z


# revision 39
# speedup vs baseline: 2397.7061x; 2397.7061x over previous
"""Distribution tokenizer (per-row 64-bin histogram) for Trainium2, 8 NeuronCores.

Problem: x (32, 512, 1024) f32, boundaries (63,) f32 sorted ascending.
For every row (b, t): bin(x) = #{j : boundaries[j] <= x} (searchsorted right),
z[b, t, k] = count of bin k in the 1024-element feature row / 1024.

Algorithm (exact, no approximations):
  For each threshold j: H_j = #{f : x[f] >= b_j}. Then with Hext =
  [F, H_0, ..., H_62, 0], counts[k] = Hext[k] - Hext[k+1], z = counts / 1024
  (division by 2^10 is exact in fp32, counts are integers <= 1024).

  The 63 thresholds are split across two engines working in parallel:
   - DVE (vector): tensor_scalar(op0=is_ge, scalar1=b_j, op1=add reduce,
     accum_out=H_j) -- one fused mask+reduce instruction per threshold.
   - ACT (scalar): pass1 s = Sign(-x + b_j) (+1 iff x < b_j, 0 iff x == b_j),
     pass2 Relu(s) with accum_out = L_j = #{x < b_j}; H_j = F - L_j.
  All comparisons are exact fp32 comparator ops against the exact boundary
  values, so ties (x == b_j) are handled identically to searchsorted.

Sharding: pure data parallel, batch dim 32 -> 8 cores x 4.
"""

import os

import numpy as np

B, T, F = 32, 512, 1024
NB = 64            # number of bins
NTH = NB - 1       # number of thresholds (63)
N_CORES = 8
ROWS_PER_CORE = (B // N_CORES) * T        # 2048
P = 128                                   # SBUF partitions
N_TILES = ROWS_PER_CORE // P              # 16

# Threshold split: j in [0, N_DVE) on the vector engine (packed pairs),
# next N_POOL on gpsimd, rest on the scalar engine (Sign+Relu 2-pass).
# HW burst-measured: DVE pair ~575ns/threshold/tile, ACT ~2.2us/threshold/tile.
N_DVE = 50
N_POOL = 0

_PROGRAM_CACHE = {}

_GE_PAIR_NAME = "GE_PAIR_ACC_ANT"


def _register_ge_pair():
    """Register a custom DVE op computing, per element,
    (x >= s0) + (x >= s1) * imm2, with accum_out = row sum.

    With imm2 = 4096 the accumulated value packs two threshold counts
    (each <= 1024 < 4096, sum < 2^22, exact in fp32) into one fused 1x
    pass -- two thresholds per 1024-element read instead of one.
    """
    from operator import add as _add

    import concourse.dve_ops as dve_ops
    from concourse.dve_spec import C0, C1, C2, Spec, Src0, lower
    from concourse.dve_uop import DveOpSpec

    if _GE_PAIR_NAME in dve_ops._SUB_OPCODE_FOR_NAME:
        for op in dve_ops.OPS:
            if op.name == _GE_PAIR_NAME:
                return op

    body = (Src0 >= C0) + (Src0 >= C1) * C2

    def ref(in0, in1, s0, s1, imm2):
        b = (
            (in0.astype(np.float32) >= s0).astype(np.float32)
            + (in0.astype(np.float32) >= s1).astype(np.float32) * imm2
        ).astype(np.float32)
        return b, b.reshape(b.shape[0], -1).sum(axis=-1, keepdims=True)

    spec = Spec(body=body, accum=_add, reference=ref)
    shas = {}
    for ver in ("v3", "v4"):
        tmp = DveOpSpec(name=_GE_PAIR_NAME, opcode=31, uops=lower(spec, ver=ver),
                        rd1_en=False)
        shas[ver] = tmp.sha(ver)
    op = dve_ops.DveOp(_GE_PAIR_NAME, spec, subdim=False, uops_sha=shas)
    dve_ops.OPS.append(op)
    dve_ops.CUSTOM_DVE_SPECS[_GE_PAIR_NAME] = spec
    dve_ops._SUB_OPCODE_FOR_NAME[_GE_PAIR_NAME] = (
        max(dve_ops._SUB_OPCODE_FOR_NAME.values()) + 1
    )
    return op


def _build_program(bvals, repeat=1):
    """Build the per-core Bass program. bvals: list of 63 exact float values.

    repeat>1 re-runs the whole tile loop (perf slope measurement only).
    """
    import concourse.bass as bass
    import concourse.mybir as mybir
    import concourse.tile as tile
    from concourse import bacc

    f32 = mybir.dt.float32
    bf16 = mybir.dt.bfloat16
    Alu = mybir.AluOpType
    Act = mybir.ActivationFunctionType

    # Bacc (not raw Bass): its compile() runs generate_event_semaphores,
    # which splits multi-wait instructions to satisfy the TRN2 limit of
    # one sync wait per instruction.
    nc = bacc.Bacc("TRN2")
    x_d = nc.dram_tensor("x", [ROWS_PER_CORE, F], f32, kind="ExternalInput")
    z_d = nc.dram_tensor("z", [ROWS_PER_CORE, NB], f32, kind="ExternalOutput")

    n_pool = N_POOL
    n_act = NTH - N_DVE - n_pool
    assert n_act >= 0 and N_DVE % 2 == 0
    n_pairs = N_DVE // 2
    ge_pair = _register_ge_pair() if n_pairs else None

    # Register const [P,1] APs for ACT bias values (boundaries used on ACT and
    # the row total F), exactly like Bass.__init__ does for 0.0/1.0. These are
    # written before the TileContext so tile scheduling sees them as plain
    # constant reads with no tracked writers.
    def register_const(value):
        key = (f32, value)
        if key not in nc.const_aps.aps:
            t = nc.alloc_sbuf_tensor(f"const-f32-{value}", [P, 1], f32)
            nc.gpsimd.memset(t.ap(), value)
            nc.const_aps.aps[key] = t.ap()

    for j in range(N_DVE + n_pool, NTH):
        register_const(bvals[j])
    register_const(float(F))
    nc.all_engine_barrier()

    with tile.TileContext(nc) as tc:
        with (
            tc.tile_pool(name="xp", bufs=3) as xp,
            tc.tile_pool(name="hp", bufs=2) as hp,
            tc.tile_pool(name="lp", bufs=2) as lp,
            tc.tile_pool(name="hp2", bufs=2) as hp2,
            tc.tile_pool(name="sp", bufs=2) as sp,
            tc.tile_pool(name="tv", bufs=2) as tv,
            tc.tile_pool(name="pp", bufs=2) as pp,
            tc.tile_pool(name="gp", bufs=2) as gp,
            tc.tile_pool(name="gp2", bufs=2) as gp2,
            tc.tile_pool(name="rp", bufs=2) as rp,
            tc.tile_pool(name="rp2", bufs=2) as rp2,
            tc.tile_pool(name="ts", bufs=2) as ts,
            tc.tile_pool(name="zp", bufs=2) as zp,
        ):
            for i in [t for _ in range(repeat) for t in range(N_TILES)]:
                xt = xp.tile([P, F], f32)
                nc.sync.dma_start(xt[:], x_d[bass.ts(i, P), :])

                # hext holds H_j * 2^-10 (pre-scaled so z is just a diff;
                # scaling integers <= 1024 by 2^-10 is exact in fp32).
                hext = hp.tile([P, NB + 1], f32)
                nc.vector.memset(hext[:, 0:1], 1.0)
                nc.vector.memset(hext[:, NB:NB + 1], 0.0)

                # DVE thresholds, packed in pairs: custom op accumulates
                # H_p + 4096*H_{p+n_pairs} in one 1x pass over the row
                # (both counts <= 1024, sum < 2^22: exact in fp32).
                # Pairing (p, p+n_pairs) keeps lo/hi unpack blocks contiguous.
                trash_v = tv.tile([P, F], f32)
                pbuf = pp.tile([P, max(n_pairs, 1)], f32)
                for p in range(n_pairs):
                    nc.vector._custom_dve(
                        ge_pair, out=trash_v[:], in0=xt[:],
                        s0=bvals[p], s1=bvals[p + n_pairs], imm2=4096.0,
                        accum_out=pbuf[:, p:p + 1],
                    )
                if n_pairs:
                    # Unpack: hi = RNE(P/4096) via the 2^23 trick (frac <=
                    # 0.25 always rounds down), lo = P - 4096*hi. Outputs are
                    # written pre-scaled by 2^-10. All steps exact in fp32.
                    rbuf = rp.tile([P, n_pairs], f32)
                    nc.vector.tensor_scalar(
                        rbuf[:], pbuf[:], float(2.0 ** -12), float(2.0 ** 23),
                        Alu.mult, Alu.add,
                    )
                    nc.vector.tensor_scalar(
                        hext[:, 1 + n_pairs:1 + 2 * n_pairs], rbuf[:],
                        float(2.0 ** 23), float(2.0 ** -10),
                        Alu.subtract, Alu.mult,
                    )
                    sbuf = rp2.tile([P, n_pairs], f32)
                    nc.vector.tensor_scalar(
                        sbuf[:], rbuf[:], float(2.0 ** 23), 4.0,
                        Alu.subtract, Alu.mult,
                    )
                    # lo*2^-10 = P*2^-10 - 4*hi
                    nc.vector.scalar_tensor_tensor(
                        hext[:, 1:1 + n_pairs], pbuf[:], float(2.0 ** -10),
                        sbuf[:], Alu.mult, Alu.subtract,
                    )

                if n_pool:
                    # GPSIMD lane: same fused is_ge+accum, on the POOL engine,
                    # into a pool-owned tile; one DVE copy moves it to hext.
                    trash_g = gp.tile([P, F], f32)
                    hpool = gp2.tile([P, n_pool], f32)
                    for k in range(n_pool):
                        j = N_DVE + k
                        nc.gpsimd.tensor_scalar(
                            trash_g[:], xt[:], bvals[j], None,
                            Alu.is_ge, Alu.add,
                            accum_out=hpool[:, k:k + 1],
                        )
                    nc.vector.tensor_scalar(
                        hext[:, 1 + N_DVE:1 + N_DVE + n_pool], hpool[:],
                        float(2.0 ** -10), None, Alu.mult,
                    )

                if n_act:
                    lbuf = lp.tile([P, n_act], f32)
                    for k in range(n_act):
                        j = N_DVE + n_pool + k
                        sgn = sp.tile([P, F], bf16)
                        nc.scalar.activation(
                            sgn[:], xt[:], Act.Sign,
                            bias=bvals[j], scale=-1.0,
                        )
                        trash_s = ts.tile([P, F], bf16)
                        nc.scalar.activation(
                            trash_s[:], sgn[:], Act.Relu,
                            accum_out=lbuf[:, k:k + 1],
                        )
                    # H_j = F - L_j, ACT-side into an ACT-owned tile; a single
                    # DVE copy then moves it into hext. Every cross-engine
                    # handoff tile has exactly one writer instruction (more
                    # blows the per-instruction sync-wait limit in codegen).
                    hact = hp2.tile([P, n_act], f32)
                    nc.scalar.activation(
                        hact[:], lbuf[:], Act.Identity,
                        bias=1.0, scale=float(-(2.0 ** -10)),
                    )
                    nc.vector.tensor_copy(
                        hext[:, 1 + N_DVE + n_pool:1 + NTH], hact[:],
                    )

                zt = zp.tile([P, NB], f32)
                nc.vector.tensor_tensor(
                    zt[:], hext[:, 0:NB], hext[:, 1:NB + 1], Alu.subtract,
                )
                nc.sync.dma_start(z_d[bass.ts(i, P), :], zt[:])

    if not nc.is_finalized():
        nc.finalize()
    return nc


def _get_program(b):
    key = b.tobytes()
    if key not in _PROGRAM_CACHE:
        _PROGRAM_CACHE[key] = _build_program([float(v) for v in b])
    return _PROGRAM_CACHE[key]


def run(x, boundaries, trace=False):
    """Run on hardware; returns (z, BassKernelResults)."""
    from concourse.bass_utils import run_bass_kernel_spmd

    x = np.ascontiguousarray(np.asarray(x), dtype=np.float32)
    b = np.ascontiguousarray(np.asarray(boundaries), dtype=np.float32)
    assert x.shape == (B, T, F) and b.shape == (NTH,)

    nc = _get_program(b)
    bpc = B // N_CORES
    in_maps = [
        {"x": np.ascontiguousarray(x[c * bpc:(c + 1) * bpc].reshape(ROWS_PER_CORE, F))}
        for c in range(N_CORES)
    ]
    res = run_bass_kernel_spmd(nc, in_maps, core_ids=list(range(N_CORES)), trace=trace)
    z = np.stack([res.results[c]["z"].reshape(bpc, T, NB) for c in range(N_CORES)])
    return z.reshape(B, T, NB), res


def kernel(x, boundaries, nr_of_bins):
    assert int(nr_of_bins) == NB
    z, _ = run(x, boundaries)
    return z
